# revision 8
# baseline (speedup 1.0000x reference)
"""BinConv2d (BatchNorm -> BinActive -> pad(-1) -> 3x3 conv) on 8 TRN2 NeuronCores.

Strategy
--------
Data-parallel over the batch dim: 32 images -> 4 per core; BN params and conv
weights replicated.

The whole BN+binactive chain collapses into a per-channel fp32 threshold U[c]
computed on the host with exact rational arithmetic so that
    x > U[c]  <=>  round(clip((x-mean)*gamma*rsqrt(var+eps)+beta, 0, 1)) == 1
bit-for-bit as XLA (cpu and neuron backends agree bitwise: the mul+add is
FMA-contracted, rsqrt is the correctly-rounded 1/sqrt).

On device each input tile needs ONE vector op:
    xb = (x > U[c]) - 0.5  in {-0.5, +0.5}, cast to fp16
and the conv weights are host-doubled (2W in fp16), so (+-0.5)*(2w) = +-w
exactly (powers of two).  The conv itself is an implicit GEMM: for each
(image, co-chunk, 8-row tile) a PSUM tile [128 co, 448] accumulates 18
matmuls (2 ci-chunks x 9 taps) with fp16 operands at full PE rate.
"""

from fractions import Fraction

import numpy as np

import concourse.bass as bass
import concourse.mybir as mybir
from concourse.bass_utils import run_bass_kernel_spmd
from concourse.tile import TileContext

N, C, H, W_ = 32, 256, 56, 56
NCORES = 8
IMGS = N // NCORES          # 4 images per core
KH = KW = 3
ROWS = 8                    # output rows per matmul tile
NRT = H // ROWS             # 7 row tiles
NB = 2 * KH * KW            # 18 accumulation steps (ci-chunk x tap)
FREE = ROWS * W_            # 448 (<=512 fp32 PSUM bank)
BN_EPS = np.float32(1e-4)

_NC = None


def _legalize_waits(nc):
    """The TRN2 ISA takes ONE sync-wait per instruction, but Tile's wait
    assignment can attach several (walrus rejects with 'Too many sync wait
    commands').  Split the extras into preceding same-engine NoOps, each
    carrying a single wait — semantically identical (engine streams are
    in-order)."""
    k = 0
    for fn in nc.m.functions:
        for blk in fn.blocks:
            new_insts = []
            for inst in blk.instructions:
                si = inst.sync_info
                waits = list(si.on_wait) if si and si.on_wait else []
                if len(waits) > 1:
                    for w in waits[:-1]:
                        nop = mybir.InstNoOp(name=f"waitsplit-{k}")
                        k += 1
                        nop.engine = inst.engine
                        nop.bass_nofuse = True
                        nop.sync_info = mybir.SyncInfo(on_wait=[w], on_update=[])
                        new_insts.append(nop)
                    inst.sync_info = mybir.SyncInfo(
                        on_wait=[waits[-1]],
                        on_update=list(si.on_update) if si.on_update else [])
                new_insts.append(inst)
            blk.instructions = new_insts


def _build_nc():
    nc = bass.Bass("TRN2")
    xs = nc.dram_tensor("xs", [IMGS, C, H, W_], mybir.dt.float32, kind="ExternalInput")
    wt = nc.dram_tensor("wt", [NB, 128, C], mybir.dt.float16, kind="ExternalInput")
    uv = nc.dram_tensor("uv", [128, 2], mybir.dt.float32, kind="ExternalInput")
    y = nc.dram_tensor("y", [IMGS, C, H, W_], mybir.dt.float32, kind="ExternalOutput")

    with TileContext(nc) as tc:
        with (
            tc.tile_pool(name="const", bufs=1) as constp,
            tc.tile_pool(name="xpb", bufs=IMGS * 2) as xpbp,
            tc.tile_pool(name="xin", bufs=3) as xinp,
            tc.tile_pool(name="out", bufs=4) as outp,
            tc.tile_pool(name="ps", bufs=4, space="PSUM") as psp,
            tc.tile_pool(name="warm", bufs=1, space="PSUM") as warmp,
        ):
            # x loads ride the SP HWDGE queue; uv/weights (and later the
            # output stores) ride the Activation HWDGE queue so startup
            # transfers run in parallel
            uv_sb = constp.tile([128, 2], mybir.dt.float32, tag="uv")
            nc.scalar.dma_start(out=uv_sb[:], in_=uv[:])
            w_sb = constp.tile([128, NB, C], mybir.dt.float16, tag="w")
            nc.scalar.dma_start(out=w_sb[:], in_=wt[:].rearrange("b p c -> p b c"))

            # binarize each (image, ci-chunk) into a -0.5-padded fp16 buffer
            xpb = [[None] * 2 for _ in range(IMGS)]
            for img in range(IMGS):
                for cc in range(2):
                    xt = xinp.tile([128, H, W_], mybir.dt.float32, tag="xin")
                    nc.sync.dma_start(out=xt[:], in_=xs[img, cc * 128:(cc + 1) * 128])

                    pb = xpbp.tile([128, H + 2, W_ + 2], mybir.dt.float16, tag="xpb")
                    nc.vector.tensor_scalar(
                        out=pb[:, 1:H + 1, 1:W_ + 1],
                        in0=xt[:],
                        scalar1=uv_sb[:, cc:cc + 1],
                        scalar2=0.5,
                        op0=mybir.AluOpType.is_gt,
                        op1=mybir.AluOpType.subtract,
                    )
                    # border pad = -0.5; on the DVE so ordering with the
                    # interior write is program-order (no semaphores)
                    nc.vector.memset(pb[:, 0, :], -0.5)
                    nc.vector.memset(pb[:, H + 1, :], -0.5)
                    nc.vector.memset(pb[:, 1:H + 1, 0], -0.5)
                    nc.vector.memset(pb[:, 1:H + 1, W_ + 1], -0.5)
                    xpb[img][cc] = pb

                if img == 0:
                    # warm the PE clock (HAM) with junk matmuls on the loaded
                    # weights while the first activations are still in flight
                    wps = warmp.tile([128, 256], mybir.dt.float32, tag="warm")
                    for i in range(32):
                        nc.tensor.matmul(
                            wps[:],
                            lhsT=w_sb[:, i % NB, 0:128],
                            rhs=w_sb[:, 0, :],
                            start=True, stop=True,
                        )

            for img in range(IMGS):
                for coj in range(2):
                    for rt in range(NRT):
                        ps = psp.tile([128, FREE], mybir.dt.float32, tag="ps")
                        for b in range(NB):
                            cc, t = divmod(b, KH * KW)
                            kh, kw = divmod(t, KW)
                            r = rt * ROWS + kh
                            nc.tensor.matmul(
                                ps[:],
                                lhsT=w_sb[:, b, coj * 128:(coj + 1) * 128],
                                rhs=xpb[img][cc][:, r:r + ROWS, kw:kw + W_],
                                start=(b == 0),
                                stop=(b == NB - 1),
                            )
                        ot = outp.tile([128, FREE], mybir.dt.float32, tag="ot")
                        nc.scalar.copy(out=ot[:], in_=ps[:])
                        nc.scalar.dma_start(
                            out=y[img, coj * 128:(coj + 1) * 128,
                                  rt * ROWS:(rt + 1) * ROWS, :],
                            in_=ot[:],
                        )
    return nc


def _get_nc():
    global _NC
    if _NC is None:
        _NC = _build_nc()
        _legalize_waits(_NC)
    return _NC


def _cr_rsqrt_f32(yv: np.float32) -> np.float32:
    """Correctly-rounded fp32 1/sqrt(y) (round-to-nearest-even) — bitwise
    identical to XLA's rsqrt on both the cpu and neuron backends."""
    fy = Fraction(float(yv))
    r0 = np.float32(1.0 / np.sqrt(float(yv)))
    cands = {float(r0)}
    lo = hi = r0
    for _ in range(2):
        lo = np.nextafter(lo, np.float32(-np.inf), dtype=np.float32)
        hi = np.nextafter(hi, np.float32(np.inf), dtype=np.float32)
        cands.update((float(lo), float(hi)))
    cands = sorted(cands)

    def gt(r):  # r > 1/sqrt(y)  <=>  r^2 * y > 1   (r > 0)
        return (Fraction(r) ** 2 * fy) > 1

    a = b = None
    for i in range(len(cands) - 1):
        if (not gt(cands[i])) and gt(cands[i + 1]):
            a, b = cands[i], cands[i + 1]
            break
    assert a is not None, "rsqrt bracket failure"
    m2 = Fraction(a + b) ** 2 * fy  # compare midpoint vs 1/sqrt(y)
    if m2 > 4:
        return np.float32(a)
    if m2 < 4:
        return np.float32(b)
    return np.float32(a) if (np.float32(a).view(np.int32) % 2 == 0) else np.float32(b)


def _thresholds(gamma, beta, running_mean, running_var) -> np.ndarray:
    """Per-channel U so that (x > U[c]) reproduces the reference's
    binarization decision bit-exactly.

    The reference (XLA, fma-contracted) binarizes +1 iff
        fl32(fma(fl32(x - mean), s, beta)) > 0.5,   s = fl32(gamma * rsqrt(var+eps))
    which, 0.5 being representable and ties rounding to even (0.5's mantissa),
    is exactly:  t1*s + beta > 1/2 + 2^-25 in exact arithmetic, t1 = fl32(x-mean).
    """
    yv = (running_var + BN_EPS).astype(np.float32)
    inv = np.array([_cr_rsqrt_f32(v) for v in yv], dtype=np.float32)
    s = (gamma * inv).astype(np.float32)
    M = Fraction(1, 2) + Fraction(1, 2 ** 25)

    U = np.zeros(C, dtype=np.float32)
    for c in range(C):
        sc, bc, mc = s[c], beta[c], running_mean[c]
        assert sc > 0, "threshold fold assumes positive BN scale"
        fs, fb = Fraction(float(sc)), Fraction(float(bc))

        def dec(xv):
            t1 = np.float32(xv) - mc
            return Fraction(float(t1)) * fs + fb > M

        xv = np.float32(np.float64(mc) + (0.5 - np.float64(bc)) / np.float64(sc))
        guard = 0
        while dec(xv):
            xv = np.nextafter(xv, np.float32(-np.inf), dtype=np.float32)
            guard += 1
            assert guard < 10000, "threshold search diverged"
        nxt = np.nextafter(xv, np.float32(np.inf), dtype=np.float32)
        while not dec(nxt):
            xv = nxt
            nxt = np.nextafter(xv, np.float32(np.inf), dtype=np.float32)
            guard += 1
            assert guard < 10000, "threshold search diverged"
        U[c] = xv  # largest fp32 x that binarizes to -1:  device does x > U
    return U


def _prep_inputs(x, gamma, beta, running_mean, running_var, W):
    U = _thresholds(
        np.asarray(gamma, dtype=np.float32),
        np.asarray(beta, dtype=np.float32),
        np.asarray(running_mean, dtype=np.float32),
        np.asarray(running_var, dtype=np.float32),
    )
    uv_dev = np.ascontiguousarray(U.reshape(2, 128).T)  # [p, cc]

    # wt[b, p, co] = fp16(2*W[co, cc*128+p, kh, kw]),  b = (cc*3 + kh)*3 + kw
    w2 = (np.asarray(W, dtype=np.float32) * np.float32(2.0)).astype(np.float16)
    wr = w2.reshape(C, 2, 128, KH, KW)
    wt_dev = np.ascontiguousarray(
        wr.transpose(1, 3, 4, 2, 0).reshape(NB, 128, C))

    x = np.ascontiguousarray(np.asarray(x, dtype=np.float32))
    in_maps = [
        {"xs": x[i * IMGS:(i + 1) * IMGS], "wt": wt_dev, "uv": uv_dev}
        for i in range(NCORES)
    ]
    return in_maps


def _run(in_maps, trace=False, **kwargs):
    return run_bass_kernel_spmd(
        _get_nc(), in_maps, list(range(NCORES)), trace=trace, **kwargs)


def kernel(x, gamma, beta, running_mean, running_var, W):
    in_maps = _prep_inputs(x, gamma, beta, running_mean, running_var, W)
    res = _run(in_maps)
    return np.concatenate([res.results[i]["y"] for i in range(NCORES)], axis=0)


# revision 11
# speedup vs baseline: 1.0302x; 1.0302x over previous
"""BinConv2d (BatchNorm -> BinActive -> pad(-1) -> 3x3 conv) on 8 TRN2 NeuronCores.

Strategy
--------
Data-parallel over the batch dim: 32 images -> 4 per core; BN params and conv
weights replicated.

The whole BN+binactive chain collapses into a per-channel fp32 threshold U[c]
computed on the host with exact rational arithmetic so that
    x > U[c]  <=>  round(clip((x-mean)*gamma*rsqrt(var+eps)+beta, 0, 1)) == 1
bit-for-bit as XLA (cpu and neuron backends agree bitwise: the mul+add is
FMA-contracted, rsqrt is the correctly-rounded 1/sqrt).

On device each input tile needs ONE vector op:
    xb = (x > U[c]) - 0.5  in {-0.5, +0.5}, cast to fp16
and the conv weights are host-doubled (2W in fp16), so (+-0.5)*(2w) = +-w
exactly (powers of two).  The conv itself is an implicit GEMM: for each
(image, co-chunk, 8-row tile) a PSUM tile [128 co, 448] accumulates 18
matmuls (2 ci-chunks x 9 taps) with fp16 operands at full PE rate.
"""

from fractions import Fraction

import numpy as np

import concourse.bass as bass
import concourse.mybir as mybir
from concourse.bass_utils import run_bass_kernel_spmd
from concourse.tile import TileContext

N, C, H, W_ = 32, 256, 56, 56
NCORES = 8
IMGS = N // NCORES          # 4 images per core
KH = KW = 3
ROWS = 8                    # output rows per matmul tile
NRT = H // ROWS             # 7 row tiles
NB = 2 * KH * KW            # 18 accumulation steps (ci-chunk x tap)
FREE = ROWS * W_            # 448 (<=512 fp32 PSUM bank)
BN_EPS = np.float32(1e-4)

_NC = None


def _legalize_waits(nc):
    """The TRN2 ISA takes ONE sync-wait per instruction, but Tile's wait
    assignment can attach several (walrus rejects with 'Too many sync wait
    commands').  Split the extras into preceding same-engine NoOps, each
    carrying a single wait — semantically identical (engine streams are
    in-order)."""
    k = 0
    for fn in nc.m.functions:
        for blk in fn.blocks:
            new_insts = []
            for inst in blk.instructions:
                si = inst.sync_info
                waits = list(si.on_wait) if si and si.on_wait else []
                if len(waits) > 1:
                    for w in waits[:-1]:
                        nop = mybir.InstNoOp(name=f"waitsplit-{k}")
                        k += 1
                        nop.engine = inst.engine
                        nop.bass_nofuse = True
                        nop.sync_info = mybir.SyncInfo(on_wait=[w], on_update=[])
                        new_insts.append(nop)
                    inst.sync_info = mybir.SyncInfo(
                        on_wait=[waits[-1]],
                        on_update=list(si.on_update) if si.on_update else [])
                new_insts.append(inst)
            blk.instructions = new_insts


def _build_nc():
    nc = bass.Bass("TRN2")
    xs = nc.dram_tensor("xs", [IMGS, C, H, W_], mybir.dt.float32, kind="ExternalInput")
    # [p, b, co] layout -> one 9216B descriptor per partition on DMA
    wt = nc.dram_tensor("wt", [128, NB, C], mybir.dt.float16, kind="ExternalInput")
    uv = nc.dram_tensor("uv", [128, 2], mybir.dt.float32, kind="ExternalInput")
    y = nc.dram_tensor("y", [IMGS, C, H, W_], mybir.dt.float32, kind="ExternalOutput")

    with TileContext(nc) as tc:
        with (
            tc.tile_pool(name="const", bufs=1) as constp,
            tc.tile_pool(name="xpb", bufs=IMGS * 2) as xpbp,
            tc.tile_pool(name="xin", bufs=3) as xinp,
            tc.tile_pool(name="out", bufs=4) as outp,
            tc.tile_pool(name="ps", bufs=4, space="PSUM") as psp,
        ):
            # x loads ride the SP HWDGE queue; uv/weights + img0's second
            # ci-chunk (and later the output stores) ride the Activation
            # HWDGE queue so the startup transfers run in parallel
            uv_sb = constp.tile([128, 2], mybir.dt.float32, tag="uv")
            nc.scalar.dma_start(out=uv_sb[:], in_=uv[:])
            w_sb = constp.tile([128, NB, C], mybir.dt.float16, tag="w")
            nc.scalar.dma_start(out=w_sb[:], in_=wt[:])

            # binarize each (image, ci-chunk) into a -0.5-padded fp16 buffer
            xpb = [[None] * 2 for _ in range(IMGS)]
            for img in range(IMGS):
                for cc in range(2):
                    xt = xinp.tile([128, H, W_], mybir.dt.float32, tag="xin")
                    dma_eng = nc.scalar if (img, cc) == (0, 1) else nc.sync
                    dma_eng.dma_start(out=xt[:], in_=xs[img, cc * 128:(cc + 1) * 128])

                    pb = xpbp.tile([128, H + 2, W_ + 2], mybir.dt.float16, tag="xpb")
                    nc.vector.tensor_scalar(
                        out=pb[:, 1:H + 1, 1:W_ + 1],
                        in0=xt[:],
                        scalar1=uv_sb[:, cc:cc + 1],
                        scalar2=0.5,
                        op0=mybir.AluOpType.is_gt,
                        op1=mybir.AluOpType.subtract,
                    )
                    # border pad = -0.5; on the DVE so ordering with the
                    # interior write is program-order (no semaphores)
                    nc.vector.memset(pb[:, 0, :], -0.5)
                    nc.vector.memset(pb[:, H + 1, :], -0.5)
                    nc.vector.memset(pb[:, 1:H + 1, 0], -0.5)
                    nc.vector.memset(pb[:, 1:H + 1, W_ + 1], -0.5)
                    xpb[img][cc] = pb

            for img in range(IMGS):
                for coj in range(2):
                    for rt in range(NRT):
                        ps = psp.tile([128, FREE], mybir.dt.float32, tag="ps")
                        for b in range(NB):
                            cc, t = divmod(b, KH * KW)
                            kh, kw = divmod(t, KW)
                            r = rt * ROWS + kh
                            nc.tensor.matmul(
                                ps[:],
                                lhsT=w_sb[:, b, coj * 128:(coj + 1) * 128],
                                rhs=xpb[img][cc][:, r:r + ROWS, kw:kw + W_],
                                start=(b == 0),
                                stop=(b == NB - 1),
                            )
                        ot = outp.tile([128, FREE], mybir.dt.float32, tag="ot")
                        nc.scalar.copy(out=ot[:], in_=ps[:])
                        nc.scalar.dma_start(
                            out=y[img, coj * 128:(coj + 1) * 128,
                                  rt * ROWS:(rt + 1) * ROWS, :],
                            in_=ot[:],
                        )
    return nc


def _get_nc():
    global _NC
    if _NC is None:
        _NC = _build_nc()
        _legalize_waits(_NC)
    return _NC


def _cr_rsqrt_f32(yv: np.float32) -> np.float32:
    """Correctly-rounded fp32 1/sqrt(y) (round-to-nearest-even) — bitwise
    identical to XLA's rsqrt on both the cpu and neuron backends."""
    fy = Fraction(float(yv))
    r0 = np.float32(1.0 / np.sqrt(float(yv)))
    cands = {float(r0)}
    lo = hi = r0
    for _ in range(2):
        lo = np.nextafter(lo, np.float32(-np.inf), dtype=np.float32)
        hi = np.nextafter(hi, np.float32(np.inf), dtype=np.float32)
        cands.update((float(lo), float(hi)))
    cands = sorted(cands)

    def gt(r):  # r > 1/sqrt(y)  <=>  r^2 * y > 1   (r > 0)
        return (Fraction(r) ** 2 * fy) > 1

    a = b = None
    for i in range(len(cands) - 1):
        if (not gt(cands[i])) and gt(cands[i + 1]):
            a, b = cands[i], cands[i + 1]
            break
    assert a is not None, "rsqrt bracket failure"
    m2 = Fraction(a + b) ** 2 * fy  # compare midpoint vs 1/sqrt(y)
    if m2 > 4:
        return np.float32(a)
    if m2 < 4:
        return np.float32(b)
    return np.float32(a) if (np.float32(a).view(np.int32) % 2 == 0) else np.float32(b)


def _thresholds(gamma, beta, running_mean, running_var) -> np.ndarray:
    """Per-channel U so that (x > U[c]) reproduces the reference's
    binarization decision bit-exactly.

    The reference (XLA, fma-contracted) binarizes +1 iff
        fl32(fma(fl32(x - mean), s, beta)) > 0.5,   s = fl32(gamma * rsqrt(var+eps))
    which, 0.5 being representable and ties rounding to even (0.5's mantissa),
    is exactly:  t1*s + beta > 1/2 + 2^-25 in exact arithmetic, t1 = fl32(x-mean).
    """
    yv = (running_var + BN_EPS).astype(np.float32)
    inv = np.array([_cr_rsqrt_f32(v) for v in yv], dtype=np.float32)
    s = (gamma * inv).astype(np.float32)
    M = Fraction(1, 2) + Fraction(1, 2 ** 25)

    U = np.zeros(C, dtype=np.float32)
    for c in range(C):
        sc, bc, mc = s[c], beta[c], running_mean[c]
        assert sc > 0, "threshold fold assumes positive BN scale"
        fs, fb = Fraction(float(sc)), Fraction(float(bc))

        def dec(xv):
            t1 = np.float32(xv) - mc
            return Fraction(float(t1)) * fs + fb > M

        xv = np.float32(np.float64(mc) + (0.5 - np.float64(bc)) / np.float64(sc))
        guard = 0
        while dec(xv):
            xv = np.nextafter(xv, np.float32(-np.inf), dtype=np.float32)
            guard += 1
            assert guard < 10000, "threshold search diverged"
        nxt = np.nextafter(xv, np.float32(np.inf), dtype=np.float32)
        while not dec(nxt):
            xv = nxt
            nxt = np.nextafter(xv, np.float32(np.inf), dtype=np.float32)
            guard += 1
            assert guard < 10000, "threshold search diverged"
        U[c] = xv  # largest fp32 x that binarizes to -1:  device does x > U
    return U


def _prep_inputs(x, gamma, beta, running_mean, running_var, W):
    U = _thresholds(
        np.asarray(gamma, dtype=np.float32),
        np.asarray(beta, dtype=np.float32),
        np.asarray(running_mean, dtype=np.float32),
        np.asarray(running_var, dtype=np.float32),
    )
    uv_dev = np.ascontiguousarray(U.reshape(2, 128).T)  # [p, cc]

    # wt[p, b, co] = fp16(2*W[co, cc*128+p, kh, kw]),  b = (cc*3 + kh)*3 + kw
    w2 = (np.asarray(W, dtype=np.float32) * np.float32(2.0)).astype(np.float16)
    wr = w2.reshape(C, 2, 128, KH, KW)
    wt_dev = np.ascontiguousarray(
        wr.transpose(2, 1, 3, 4, 0).reshape(128, NB, C))

    x = np.ascontiguousarray(np.asarray(x, dtype=np.float32))
    in_maps = [
        {"xs": x[i * IMGS:(i + 1) * IMGS], "wt": wt_dev, "uv": uv_dev}
        for i in range(NCORES)
    ]
    return in_maps


def _run(in_maps, trace=False, **kwargs):
    return run_bass_kernel_spmd(
        _get_nc(), in_maps, list(range(NCORES)), trace=trace, **kwargs)


def kernel(x, gamma, beta, running_mean, running_var, W):
    in_maps = _prep_inputs(x, gamma, beta, running_mean, running_var, W)
    res = _run(in_maps)
    return np.concatenate([res.results[i]["y"] for i in range(NCORES)], axis=0)


# revision 15
# speedup vs baseline: 1.0703x; 1.0389x over previous
"""BinConv2d (BatchNorm -> BinActive -> pad(-1) -> 3x3 conv) on 8 TRN2 NeuronCores.

Strategy
--------
Data-parallel over the batch dim: 32 images -> 4 per core; BN params and conv
weights replicated.

The whole BN+binactive chain collapses into a per-channel fp32 threshold U[c]
computed on the host with exact rational arithmetic so that
    x > U[c]  <=>  round(clip((x-mean)*gamma*rsqrt(var+eps)+beta, 0, 1)) == 1
bit-for-bit as XLA (cpu and neuron backends agree bitwise: the mul+add is
FMA-contracted, rsqrt is the correctly-rounded 1/sqrt).

On device each input tile needs ONE vector op:
    xb = (x > U[c]) - 0.5  in {-0.5, +0.5}, cast to fp16
and the conv weights are host-doubled (2W in fp16), so (+-0.5)*(2w) = +-w
exactly (powers of two).  The conv itself is an implicit GEMM: for each
(image, co-chunk, 8-row tile) a PSUM tile [128 co, 448] accumulates 18
matmuls (2 ci-chunks x 9 taps) with fp16 operands at full PE rate.
"""

from fractions import Fraction

import numpy as np

import concourse.bass as bass
import concourse.mybir as mybir
from concourse.bass_utils import run_bass_kernel_spmd
from concourse.tile import TileContext

N, C, H, W_ = 32, 256, 56, 56
NCORES = 8
IMGS = N // NCORES          # 4 images per core
KH = KW = 3
ROWS = 8                    # output rows per matmul tile
NRT = H // ROWS             # 7 row tiles
NB = 2 * KH * KW            # 18 accumulation steps (ci-chunk x tap)
FREE = ROWS * W_            # 448 (<=512 fp32 PSUM bank)
BN_EPS = np.float32(1e-4)

_NC = None


def _legalize_waits(nc):
    """The TRN2 ISA takes ONE sync-wait per instruction, but Tile's wait
    assignment can attach several (walrus rejects with 'Too many sync wait
    commands').  Split the extras into preceding same-engine NoOps, each
    carrying a single wait — semantically identical (engine streams are
    in-order)."""
    k = 0
    for fn in nc.m.functions:
        for blk in fn.blocks:
            new_insts = []
            for inst in blk.instructions:
                si = inst.sync_info
                waits = list(si.on_wait) if si and si.on_wait else []
                if len(waits) > 1:
                    for w in waits[:-1]:
                        nop = mybir.InstNoOp(name=f"waitsplit-{k}")
                        k += 1
                        nop.engine = inst.engine
                        nop.bass_nofuse = True
                        nop.sync_info = mybir.SyncInfo(on_wait=[w], on_update=[])
                        new_insts.append(nop)
                    inst.sync_info = mybir.SyncInfo(
                        on_wait=[waits[-1]],
                        on_update=list(si.on_update) if si.on_update else [])
                new_insts.append(inst)
            blk.instructions = new_insts


def _build_nc():
    nc = bass.Bass("TRN2")
    xs = nc.dram_tensor("xs", [IMGS, C, H, W_], mybir.dt.float32, kind="ExternalInput")
    # weights [p, b*co] with the U thresholds bit-packed at the tail ->
    # ONE 9224B descriptor per partition for all constants
    cw = nc.dram_tensor("cw", [128, NB * C + 4], mybir.dt.float16,
                        kind="ExternalInput")
    y = nc.dram_tensor("y", [IMGS, C, H, W_], mybir.dt.float32, kind="ExternalOutput")

    with TileContext(nc) as tc:
        with (
            tc.tile_pool(name="const", bufs=1) as constp,
            tc.tile_pool(name="xpb", bufs=IMGS * 2) as xpbp,
            tc.tile_pool(name="xin", bufs=3) as xinp,
            tc.tile_pool(name="out", bufs=4) as outp,
            tc.tile_pool(name="ps", bufs=4, space="PSUM") as psp,
        ):
            cw_sb = constp.tile([128, NB * C + 4], mybir.dt.float16, tag="cw")
            nc.sync.dma_start(out=cw_sb[:], in_=cw[:])
            w_sb = cw_sb[:, :NB * C].rearrange("p (b c) -> p b c", c=C)
            uv_sb = cw_sb[:, NB * C:NB * C + 4].bitcast(mybir.dt.float32)

            def binarize(pb, xt_sl, cc, r0, r1):
                nc.vector.tensor_scalar(
                    out=pb[:, 1 + r0:1 + r1, 1:W_ + 1],
                    in0=xt_sl,
                    scalar1=uv_sb[:, cc:cc + 1],
                    scalar2=0.5,
                    op0=mybir.AluOpType.is_gt,
                    op1=mybir.AluOpType.subtract,
                )

            def borders(pb):
                # border pad = -0.5; on the DVE so ordering with the
                # interior write is program-order (no semaphores)
                nc.vector.memset(pb[:, 0, :], -0.5)
                nc.vector.memset(pb[:, H + 1, :], -0.5)
                nc.vector.memset(pb[:, 1:H + 1, 0], -0.5)
                nc.vector.memset(pb[:, 1:H + 1, W_ + 1], -0.5)

            # binarize each (image, ci-chunk) into a -0.5-padded fp16 buffer.
            # img0 streams in 8-row chunks (alternating ci-chunks) so the
            # first matmul group can start long before the full image lands.
            xpb = [[None] * 2 for _ in range(IMGS)]
            img0 = []
            for cc in range(2):
                xt = xinp.tile([128, H, W_], mybir.dt.float32, tag="xin")
                pb = xpbp.tile([128, H + 2, W_ + 2], mybir.dt.float16, tag="xpb")
                borders(pb)
                img0.append((xt, pb))
                xpb[0][cc] = pb
            for k in range(NRT):
                r0, r1 = k * ROWS, (k + 1) * ROWS
                for cc, (xt, pb) in enumerate(img0):
                    nc.sync.dma_start(
                        out=xt[:, r0:r1, :],
                        in_=xs[0, cc * 128:(cc + 1) * 128, r0:r1, :])
                    binarize(pb, xt[:, r0:r1, :], cc, r0, r1)

            for img in range(1, IMGS):
                for cc in range(2):
                    xt = xinp.tile([128, H, W_], mybir.dt.float32, tag="xin")
                    nc.sync.dma_start(out=xt[:], in_=xs[img, cc * 128:(cc + 1) * 128])
                    pb = xpbp.tile([128, H + 2, W_ + 2], mybir.dt.float16, tag="xpb")
                    binarize(pb, xt[:], cc, 0, H)
                    borders(pb)
                    xpb[img][cc] = pb

            for img in range(IMGS):
                for coj in range(2):
                    for rt in range(NRT):
                        ps = psp.tile([128, FREE], mybir.dt.float32, tag="ps")
                        for b in range(NB):
                            cc, t = divmod(b, KH * KW)
                            kh, kw = divmod(t, KW)
                            r = rt * ROWS + kh
                            nc.tensor.matmul(
                                ps[:],
                                lhsT=w_sb[:, b, coj * 128:(coj + 1) * 128],
                                rhs=xpb[img][cc][:, r:r + ROWS, kw:kw + W_],
                                start=(b == 0),
                                stop=(b == NB - 1),
                            )
                        ot = outp.tile([128, FREE], mybir.dt.float32, tag="ot")
                        nc.scalar.copy(out=ot[:], in_=ps[:])
                        nc.sync.dma_start(
                            out=y[img, coj * 128:(coj + 1) * 128,
                                  rt * ROWS:(rt + 1) * ROWS, :],
                            in_=ot[:],
                        )
    return nc


def _get_nc():
    global _NC
    if _NC is None:
        _NC = _build_nc()
        _legalize_waits(_NC)
    return _NC


def _cr_rsqrt_f32(yv: np.float32) -> np.float32:
    """Correctly-rounded fp32 1/sqrt(y) (round-to-nearest-even) — bitwise
    identical to XLA's rsqrt on both the cpu and neuron backends."""
    fy = Fraction(float(yv))
    r0 = np.float32(1.0 / np.sqrt(float(yv)))
    cands = {float(r0)}
    lo = hi = r0
    for _ in range(2):
        lo = np.nextafter(lo, np.float32(-np.inf), dtype=np.float32)
        hi = np.nextafter(hi, np.float32(np.inf), dtype=np.float32)
        cands.update((float(lo), float(hi)))
    cands = sorted(cands)

    def gt(r):  # r > 1/sqrt(y)  <=>  r^2 * y > 1   (r > 0)
        return (Fraction(r) ** 2 * fy) > 1

    a = b = None
    for i in range(len(cands) - 1):
        if (not gt(cands[i])) and gt(cands[i + 1]):
            a, b = cands[i], cands[i + 1]
            break
    assert a is not None, "rsqrt bracket failure"
    m2 = Fraction(a + b) ** 2 * fy  # compare midpoint vs 1/sqrt(y)
    if m2 > 4:
        return np.float32(a)
    if m2 < 4:
        return np.float32(b)
    return np.float32(a) if (np.float32(a).view(np.int32) % 2 == 0) else np.float32(b)


def _thresholds(gamma, beta, running_mean, running_var) -> np.ndarray:
    """Per-channel U so that (x > U[c]) reproduces the reference's
    binarization decision bit-exactly.

    The reference (XLA, fma-contracted) binarizes +1 iff
        fl32(fma(fl32(x - mean), s, beta)) > 0.5,   s = fl32(gamma * rsqrt(var+eps))
    which, 0.5 being representable and ties rounding to even (0.5's mantissa),
    is exactly:  t1*s + beta > 1/2 + 2^-25 in exact arithmetic, t1 = fl32(x-mean).
    """
    yv = (running_var + BN_EPS).astype(np.float32)
    inv = np.array([_cr_rsqrt_f32(v) for v in yv], dtype=np.float32)
    s = (gamma * inv).astype(np.float32)
    M = Fraction(1, 2) + Fraction(1, 2 ** 25)

    U = np.zeros(C, dtype=np.float32)
    for c in range(C):
        sc, bc, mc = s[c], beta[c], running_mean[c]
        assert sc > 0, "threshold fold assumes positive BN scale"
        fs, fb = Fraction(float(sc)), Fraction(float(bc))

        def dec(xv):
            t1 = np.float32(xv) - mc
            return Fraction(float(t1)) * fs + fb > M

        xv = np.float32(np.float64(mc) + (0.5 - np.float64(bc)) / np.float64(sc))
        guard = 0
        while dec(xv):
            xv = np.nextafter(xv, np.float32(-np.inf), dtype=np.float32)
            guard += 1
            assert guard < 10000, "threshold search diverged"
        nxt = np.nextafter(xv, np.float32(np.inf), dtype=np.float32)
        while not dec(nxt):
            xv = nxt
            nxt = np.nextafter(xv, np.float32(np.inf), dtype=np.float32)
            guard += 1
            assert guard < 10000, "threshold search diverged"
        U[c] = xv  # largest fp32 x that binarizes to -1:  device does x > U
    return U


def _prep_inputs(x, gamma, beta, running_mean, running_var, W):
    U = _thresholds(
        np.asarray(gamma, dtype=np.float32),
        np.asarray(beta, dtype=np.float32),
        np.asarray(running_mean, dtype=np.float32),
        np.asarray(running_var, dtype=np.float32),
    )
    uv_dev = np.ascontiguousarray(U.reshape(2, 128).T)  # [p, cc]

    # wt[p, b, co] = fp16(2*W[co, cc*128+p, kh, kw]),  b = (cc*3 + kh)*3 + kw
    w2 = (np.asarray(W, dtype=np.float32) * np.float32(2.0)).astype(np.float16)
    wr = w2.reshape(C, 2, 128, KH, KW)
    wt_dev = wr.transpose(2, 1, 3, 4, 0).reshape(128, NB * C)
    # one combined constant tensor: weights ++ bit-packed U thresholds
    cw_dev = np.ascontiguousarray(
        np.concatenate([wt_dev, uv_dev.view(np.float16)], axis=1))

    x = np.ascontiguousarray(np.asarray(x, dtype=np.float32))
    in_maps = [
        {"xs": x[i * IMGS:(i + 1) * IMGS], "cw": cw_dev}
        for i in range(NCORES)
    ]
    return in_maps


def _run(in_maps, trace=False, **kwargs):
    return run_bass_kernel_spmd(
        _get_nc(), in_maps, list(range(NCORES)), trace=trace, **kwargs)


def kernel(x, gamma, beta, running_mean, running_var, W):
    in_maps = _prep_inputs(x, gamma, beta, running_mean, running_var, W)
    res = _run(in_maps)
    return np.concatenate([res.results[i]["y"] for i in range(NCORES)], axis=0)


# revision 17
# speedup vs baseline: 1.1110x; 1.0380x over previous
"""BinConv2d (BatchNorm -> BinActive -> pad(-1) -> 3x3 conv) on 8 TRN2 NeuronCores.

Strategy
--------
Data-parallel over the batch dim: 32 images -> 4 per core; BN params and conv
weights replicated.

The whole BN+binactive chain collapses into a per-channel fp32 threshold U[c]
computed on the host with exact rational arithmetic so that
    x > U[c]  <=>  round(clip((x-mean)*gamma*rsqrt(var+eps)+beta, 0, 1)) == 1
bit-for-bit as XLA (cpu and neuron backends agree bitwise: the mul+add is
FMA-contracted, rsqrt is the correctly-rounded 1/sqrt).

On device each input tile needs ONE vector op:
    xb = (x > U[c]) - 0.5  in {-0.5, +0.5}, cast to fp16
and the conv weights are host-doubled (2W in fp16), so (+-0.5)*(2w) = +-w
exactly (powers of two).  The conv itself is an implicit GEMM: for each
(image, co-chunk, 8-row tile) a PSUM tile [128 co, 448] accumulates 18
matmuls (2 ci-chunks x 9 taps) with fp16 operands at full PE rate.
"""

from fractions import Fraction

import numpy as np

import concourse.bass as bass
import concourse.mybir as mybir
from concourse.bass_utils import run_bass_kernel_spmd
from concourse.tile import TileContext

N, C, H, W_ = 32, 256, 56, 56
NCORES = 8
IMGS = N // NCORES          # 4 images per core
KH = KW = 3
ROWS = 8                    # output rows per matmul tile
NRT = H // ROWS             # 7 row tiles
NB = 2 * KH * KW            # 18 accumulation steps (ci-chunk x tap)
FREE = ROWS * W_            # 448 (<=512 fp32 PSUM bank)
BN_EPS = np.float32(1e-4)

_NC = None


def _legalize_waits(nc):
    """The TRN2 ISA takes ONE sync-wait per instruction, but Tile's wait
    assignment can attach several (walrus rejects with 'Too many sync wait
    commands').  Split the extras into preceding same-engine NoOps, each
    carrying a single wait — semantically identical (engine streams are
    in-order)."""
    k = 0
    for fn in nc.m.functions:
        for blk in fn.blocks:
            new_insts = []
            for inst in blk.instructions:
                si = inst.sync_info
                waits = list(si.on_wait) if si and si.on_wait else []
                if len(waits) > 1:
                    for w in waits[:-1]:
                        nop = mybir.InstNoOp(name=f"waitsplit-{k}")
                        k += 1
                        nop.engine = inst.engine
                        nop.bass_nofuse = True
                        nop.sync_info = mybir.SyncInfo(on_wait=[w], on_update=[])
                        new_insts.append(nop)
                    inst.sync_info = mybir.SyncInfo(
                        on_wait=[waits[-1]],
                        on_update=list(si.on_update) if si.on_update else [])
                new_insts.append(inst)
            blk.instructions = new_insts


def _build_nc():
    nc = bass.Bass("TRN2")
    xs = nc.dram_tensor("xs", [IMGS, C, H, W_], mybir.dt.float32, kind="ExternalInput")
    # weights [p, b*co] with the U thresholds bit-packed at the tail ->
    # ONE 9224B descriptor per partition for all constants
    cw = nc.dram_tensor("cw", [128, NB * C + 4], mybir.dt.float16,
                        kind="ExternalInput")
    y = nc.dram_tensor("y", [IMGS, C, H, W_], mybir.dt.float32, kind="ExternalOutput")

    with TileContext(nc) as tc:
        with (
            tc.tile_pool(name="const", bufs=1) as constp,
            tc.tile_pool(name="xpb", bufs=IMGS * 2) as xpbp,
            tc.tile_pool(name="xin", bufs=4) as xinp,
            tc.tile_pool(name="out", bufs=6) as outp,
            tc.tile_pool(name="ps", bufs=4, space="PSUM") as psp,
            tc.tile_pool(name="warm", bufs=1, space="PSUM") as warmp,
        ):
            cw_sb = constp.tile([128, NB * C + 4], mybir.dt.float16, tag="cw")
            nc.sync.dma_start(out=cw_sb[:], in_=cw[:])
            w_sb = cw_sb[:, :NB * C].rearrange("p (b c) -> p b c", c=C)
            uv_sb = cw_sb[:, NB * C:NB * C + 4].bitcast(mybir.dt.float32)

            def binarize(pb, xt_sl, cc, r0, r1):
                nc.vector.tensor_scalar(
                    out=pb[:, 1 + r0:1 + r1, 1:W_ + 1],
                    in0=xt_sl,
                    scalar1=uv_sb[:, cc:cc + 1],
                    scalar2=0.5,
                    op0=mybir.AluOpType.is_gt,
                    op1=mybir.AluOpType.subtract,
                )

            def borders(pb):
                # border pad = -0.5; on the DVE so ordering with the
                # interior write is program-order (no semaphores).
                # two strided memsets: rows {0,57}, then cols {0,57}
                nc.vector.memset(pb[:, 0:H + 2:H + 1, :], -0.5)
                nc.vector.memset(pb[:, 1:H + 1, 0:W_ + 2:W_ + 1], -0.5)

            # binarize each (image, ci-chunk) into a -0.5-padded fp16 buffer.
            # img0 streams in 8-row chunks (alternating ci-chunks) so the
            # first matmul group can start long before the full image lands.
            xpb = [[None] * 2 for _ in range(IMGS)]
            img0 = []
            for cc in range(2):
                xt = xinp.tile([128, H, W_], mybir.dt.float32, tag="xin")
                pb = xpbp.tile([128, H + 2, W_ + 2], mybir.dt.float16, tag="xpb")
                borders(pb)
                img0.append((xt, pb))
                xpb[0][cc] = pb
            for k in range(NRT):
                r0, r1 = k * ROWS, (k + 1) * ROWS
                for cc, (xt, pb) in enumerate(img0):
                    nc.sync.dma_start(
                        out=xt[:, r0:r1, :],
                        in_=xs[0, cc * 128:(cc + 1) * 128, r0:r1, :])
                    binarize(pb, xt[:, r0:r1, :], cc, r0, r1)

            # warm the PE clock (HAM) on junk matmuls in the otherwise-dead
            # window between the const load and the first real matmul
            wps = warmp.tile([128, C], mybir.dt.float32, tag="warm")
            for i in range(16):
                nc.tensor.matmul(wps[:], lhsT=w_sb[:, i % NB, 0:128],
                                 rhs=w_sb[:, 0, :], start=True, stop=True)

            def load_img(img):
                for cc in range(2):
                    xt = xinp.tile([128, H, W_], mybir.dt.float32, tag="xin")
                    nc.sync.dma_start(out=xt[:], in_=xs[img, cc * 128:(cc + 1) * 128])
                    pb = xpbp.tile([128, H + 2, W_ + 2], mybir.dt.float16, tag="xpb")
                    binarize(pb, xt[:], cc, 0, H)
                    borders(pb)
                    xpb[img][cc] = pb

            for img in range(IMGS):
                # emit the next image's load+binarize first so its DMAs are
                # enqueued ahead of this image's output stores
                if img + 1 < IMGS:
                    load_img(img + 1)
                for coj in range(2):
                    for rt in range(NRT):
                        ps = psp.tile([128, FREE], mybir.dt.float32, tag="ps")
                        for b in range(NB):
                            cc, t = divmod(b, KH * KW)
                            kh, kw = divmod(t, KW)
                            r = rt * ROWS + kh
                            nc.tensor.matmul(
                                ps[:],
                                lhsT=w_sb[:, b, coj * 128:(coj + 1) * 128],
                                rhs=xpb[img][cc][:, r:r + ROWS, kw:kw + W_],
                                start=(b == 0),
                                stop=(b == NB - 1),
                            )
                        ot = outp.tile([128, FREE], mybir.dt.float32, tag="ot")
                        nc.scalar.copy(out=ot[:], in_=ps[:])
                        nc.sync.dma_start(
                            out=y[img, coj * 128:(coj + 1) * 128,
                                  rt * ROWS:(rt + 1) * ROWS, :],
                            in_=ot[:],
                        )
    return nc


def _get_nc():
    global _NC
    if _NC is None:
        _NC = _build_nc()
        _legalize_waits(_NC)
    return _NC


def _cr_rsqrt_f32(yv: np.float32) -> np.float32:
    """Correctly-rounded fp32 1/sqrt(y) (round-to-nearest-even) — bitwise
    identical to XLA's rsqrt on both the cpu and neuron backends."""
    fy = Fraction(float(yv))
    r0 = np.float32(1.0 / np.sqrt(float(yv)))
    cands = {float(r0)}
    lo = hi = r0
    for _ in range(2):
        lo = np.nextafter(lo, np.float32(-np.inf), dtype=np.float32)
        hi = np.nextafter(hi, np.float32(np.inf), dtype=np.float32)
        cands.update((float(lo), float(hi)))
    cands = sorted(cands)

    def gt(r):  # r > 1/sqrt(y)  <=>  r^2 * y > 1   (r > 0)
        return (Fraction(r) ** 2 * fy) > 1

    a = b = None
    for i in range(len(cands) - 1):
        if (not gt(cands[i])) and gt(cands[i + 1]):
            a, b = cands[i], cands[i + 1]
            break
    assert a is not None, "rsqrt bracket failure"
    m2 = Fraction(a + b) ** 2 * fy  # compare midpoint vs 1/sqrt(y)
    if m2 > 4:
        return np.float32(a)
    if m2 < 4:
        return np.float32(b)
    return np.float32(a) if (np.float32(a).view(np.int32) % 2 == 0) else np.float32(b)


def _thresholds(gamma, beta, running_mean, running_var) -> np.ndarray:
    """Per-channel U so that (x > U[c]) reproduces the reference's
    binarization decision bit-exactly.

    The reference (XLA, fma-contracted) binarizes +1 iff
        fl32(fma(fl32(x - mean), s, beta)) > 0.5,   s = fl32(gamma * rsqrt(var+eps))
    which, 0.5 being representable and ties rounding to even (0.5's mantissa),
    is exactly:  t1*s + beta > 1/2 + 2^-25 in exact arithmetic, t1 = fl32(x-mean).
    """
    yv = (running_var + BN_EPS).astype(np.float32)
    inv = np.array([_cr_rsqrt_f32(v) for v in yv], dtype=np.float32)
    s = (gamma * inv).astype(np.float32)
    M = Fraction(1, 2) + Fraction(1, 2 ** 25)

    U = np.zeros(C, dtype=np.float32)
    for c in range(C):
        sc, bc, mc = s[c], beta[c], running_mean[c]
        assert sc > 0, "threshold fold assumes positive BN scale"
        fs, fb = Fraction(float(sc)), Fraction(float(bc))

        def dec(xv):
            t1 = np.float32(xv) - mc
            return Fraction(float(t1)) * fs + fb > M

        xv = np.float32(np.float64(mc) + (0.5 - np.float64(bc)) / np.float64(sc))
        guard = 0
        while dec(xv):
            xv = np.nextafter(xv, np.float32(-np.inf), dtype=np.float32)
            guard += 1
            assert guard < 10000, "threshold search diverged"
        nxt = np.nextafter(xv, np.float32(np.inf), dtype=np.float32)
        while not dec(nxt):
            xv = nxt
            nxt = np.nextafter(xv, np.float32(np.inf), dtype=np.float32)
            guard += 1
            assert guard < 10000, "threshold search diverged"
        U[c] = xv  # largest fp32 x that binarizes to -1:  device does x > U
    return U


def _prep_inputs(x, gamma, beta, running_mean, running_var, W):
    U = _thresholds(
        np.asarray(gamma, dtype=np.float32),
        np.asarray(beta, dtype=np.float32),
        np.asarray(running_mean, dtype=np.float32),
        np.asarray(running_var, dtype=np.float32),
    )
    uv_dev = np.ascontiguousarray(U.reshape(2, 128).T)  # [p, cc]

    # wt[p, b, co] = fp16(2*W[co, cc*128+p, kh, kw]),  b = (cc*3 + kh)*3 + kw
    w2 = (np.asarray(W, dtype=np.float32) * np.float32(2.0)).astype(np.float16)
    wr = w2.reshape(C, 2, 128, KH, KW)
    wt_dev = wr.transpose(2, 1, 3, 4, 0).reshape(128, NB * C)
    # one combined constant tensor: weights ++ bit-packed U thresholds
    cw_dev = np.ascontiguousarray(
        np.concatenate([wt_dev, uv_dev.view(np.float16)], axis=1))

    x = np.ascontiguousarray(np.asarray(x, dtype=np.float32))
    in_maps = [
        {"xs": x[i * IMGS:(i + 1) * IMGS], "cw": cw_dev}
        for i in range(NCORES)
    ]
    return in_maps


def _run(in_maps, trace=False, **kwargs):
    return run_bass_kernel_spmd(
        _get_nc(), in_maps, list(range(NCORES)), trace=trace, **kwargs)


def kernel(x, gamma, beta, running_mean, running_var, W):
    in_maps = _prep_inputs(x, gamma, beta, running_mean, running_var, W)
    res = _run(in_maps)
    return np.concatenate([res.results[i]["y"] for i in range(NCORES)], axis=0)


# revision 21
# speedup vs baseline: 1.1205x; 1.0086x over previous
"""BinConv2d (BatchNorm -> BinActive -> pad(-1) -> 3x3 conv) on 8 TRN2 NeuronCores.

Strategy
--------
Data-parallel over the batch dim: 32 images -> 4 per core; BN params and conv
weights replicated.

The whole BN+binactive chain collapses into a per-channel fp32 threshold U[c]
computed on the host with exact rational arithmetic so that
    x > U[c]  <=>  round(clip((x-mean)*gamma*rsqrt(var+eps)+beta, 0, 1)) == 1
bit-for-bit as XLA (cpu and neuron backends agree bitwise: the mul+add is
FMA-contracted, rsqrt is the correctly-rounded 1/sqrt).

On device each input tile needs ONE vector op:
    xb = (x > U[c]) - 0.5  in {-0.5, +0.5}, cast to fp16
and the conv weights are host-doubled (2W in fp16), so (+-0.5)*(2w) = +-w
exactly (powers of two).  The conv itself is an implicit GEMM: for each
(image, co-chunk, 8-row tile) a PSUM tile [128 co, 448] accumulates 18
matmuls (2 ci-chunks x 9 taps) with fp16 operands at full PE rate.
"""

from fractions import Fraction

import numpy as np

import concourse.bass as bass
import concourse.mybir as mybir
from concourse.bass_utils import run_bass_kernel_spmd
from concourse.tile import TileContext

N, C, H, W_ = 32, 256, 56, 56
NCORES = 8
IMGS = N // NCORES          # 4 images per core
KH = KW = 3
ROWS = 8                    # output rows per matmul tile
NRT = H // ROWS             # 7 row tiles
NB = 2 * KH * KW            # 18 accumulation steps (ci-chunk x tap)
FREE = ROWS * W_            # 448 (<=512 fp32 PSUM bank)
BN_EPS = np.float32(1e-4)

_NC = None


def _legalize_waits(nc):
    """The TRN2 ISA takes ONE sync-wait per instruction, but Tile's wait
    assignment can attach several (walrus rejects with 'Too many sync wait
    commands').  Split the extras into preceding same-engine NoOps, each
    carrying a single wait — semantically identical (engine streams are
    in-order)."""
    k = 0
    for fn in nc.m.functions:
        for blk in fn.blocks:
            new_insts = []
            for inst in blk.instructions:
                si = inst.sync_info
                waits = list(si.on_wait) if si and si.on_wait else []
                if len(waits) > 1:
                    for w in waits[:-1]:
                        nop = mybir.InstNoOp(name=f"waitsplit-{k}")
                        k += 1
                        nop.engine = inst.engine
                        nop.bass_nofuse = True
                        nop.sync_info = mybir.SyncInfo(on_wait=[w], on_update=[])
                        new_insts.append(nop)
                    inst.sync_info = mybir.SyncInfo(
                        on_wait=[waits[-1]],
                        on_update=list(si.on_update) if si.on_update else [])
                new_insts.append(inst)
            blk.instructions = new_insts


def _build_nc():
    nc = bass.Bass("TRN2")
    xs = nc.dram_tensor("xs", [IMGS, C, H, W_], mybir.dt.float32, kind="ExternalInput")
    # weights [p, b*co] with the U thresholds bit-packed at the tail ->
    # ONE 9224B descriptor per partition for all constants
    cw = nc.dram_tensor("cw", [128, NB * C + 4], mybir.dt.float16,
                        kind="ExternalInput")
    y = nc.dram_tensor("y", [IMGS, C, H, W_], mybir.dt.float32, kind="ExternalOutput")

    with TileContext(nc) as tc:
        with (
            tc.tile_pool(name="const", bufs=1) as constp,
            tc.tile_pool(name="xpb", bufs=IMGS * 2) as xpbp,
            tc.tile_pool(name="xin", bufs=4) as xinp,
            tc.tile_pool(name="out", bufs=6) as outp,
            tc.tile_pool(name="ps", bufs=6, space="PSUM") as psp,
            tc.tile_pool(name="warm", bufs=1, space="PSUM") as warmp,
        ):
            # warm the PE clock (HAM) on a dependency-free junk tile so the
            # ramp happens during the framework preamble, not on the
            # critical path behind the weight DMA
            junk = constp.tile([128, 448], mybir.dt.float16, tag="junk")
            nc.gpsimd.memset(junk[:], 0.25)
            wps = warmp.tile([128, 448], mybir.dt.float32, tag="warm")
            for i in range(16):
                nc.tensor.matmul(wps[:], lhsT=junk[:, 0:128], rhs=junk[:],
                                 start=True, stop=True)

            cw_sb = constp.tile([128, NB * C + 4], mybir.dt.float16, tag="cw")
            nc.sync.dma_start(out=cw_sb[:], in_=cw[:])
            w_sb = cw_sb[:, :NB * C].rearrange("p (b c) -> p b c", c=C)
            uv_sb = cw_sb[:, NB * C:NB * C + 4].bitcast(mybir.dt.float32)

            def binarize(pb, xt_sl, cc, r0, r1):
                nc.vector.tensor_scalar(
                    out=pb[:, 1 + r0:1 + r1, 1:W_ + 1],
                    in0=xt_sl,
                    scalar1=uv_sb[:, cc:cc + 1],
                    scalar2=0.5,
                    op0=mybir.AluOpType.is_gt,
                    op1=mybir.AluOpType.subtract,
                )

            def borders(pb):
                # border pad = -0.5; on the DVE so ordering with the
                # interior write is program-order (no semaphores).
                # two strided memsets: rows {0,57}, then cols {0,57}
                nc.vector.memset(pb[:, 0:H + 2:H + 1, :], -0.5)
                nc.vector.memset(pb[:, 1:H + 1, 0:W_ + 2:W_ + 1], -0.5)

            # binarize each (image, ci-chunk) into a -0.5-padded fp16 buffer.
            # img0 streams in 8-row chunks (alternating ci-chunks) so the
            # first matmul group can start long before the full image lands.
            xpb = [[None] * 2 for _ in range(IMGS)]
            img0 = []
            for cc in range(2):
                xt = xinp.tile([128, H, W_], mybir.dt.float32, tag="xin")
                pb = xpbp.tile([128, H + 2, W_ + 2], mybir.dt.float16, tag="xpb")
                borders(pb)
                img0.append((xt, pb))
                xpb[0][cc] = pb
            for k in range(NRT):
                r0, r1 = k * ROWS, (k + 1) * ROWS
                for cc, (xt, pb) in enumerate(img0):
                    nc.sync.dma_start(
                        out=xt[:, r0:r1, :],
                        in_=xs[0, cc * 128:(cc + 1) * 128, r0:r1, :])
                    binarize(pb, xt[:, r0:r1, :], cc, r0, r1)

            def load_img(img):
                for cc in range(2):
                    xt = xinp.tile([128, H, W_], mybir.dt.float32, tag="xin")
                    nc.sync.dma_start(out=xt[:], in_=xs[img, cc * 128:(cc + 1) * 128])
                    pb = xpbp.tile([128, H + 2, W_ + 2], mybir.dt.float16, tag="xpb")
                    binarize(pb, xt[:], cc, 0, H)
                    borders(pb)
                    xpb[img][cc] = pb

            for img in range(IMGS):
                # emit the next image's load+binarize first so its DMAs are
                # enqueued ahead of this image's output stores
                if img + 1 < IMGS:
                    load_img(img + 1)
                for coj in range(2):
                    for rt in range(NRT):
                        ps = psp.tile([128, FREE], mybir.dt.float32, tag="ps")
                        for b in range(NB):
                            cc, t = divmod(b, KH * KW)
                            kh, kw = divmod(t, KW)
                            r = rt * ROWS + kh
                            nc.tensor.matmul(
                                ps[:],
                                lhsT=w_sb[:, b, coj * 128:(coj + 1) * 128],
                                rhs=xpb[img][cc][:, r:r + ROWS, kw:kw + W_],
                                start=(b == 0),
                                stop=(b == NB - 1),
                            )
                        ot = outp.tile([128, FREE], mybir.dt.float32, tag="ot")
                        nc.scalar.copy(out=ot[:], in_=ps[:])
                        nc.sync.dma_start(
                            out=y[img, coj * 128:(coj + 1) * 128,
                                  rt * ROWS:(rt + 1) * ROWS, :],
                            in_=ot[:],
                        )
    return nc


def _get_nc():
    global _NC
    if _NC is None:
        _NC = _build_nc()
        _legalize_waits(_NC)
    return _NC


def _cr_rsqrt_f32(yv: np.float32) -> np.float32:
    """Correctly-rounded fp32 1/sqrt(y) (round-to-nearest-even) — bitwise
    identical to XLA's rsqrt on both the cpu and neuron backends."""
    fy = Fraction(float(yv))
    r0 = np.float32(1.0 / np.sqrt(float(yv)))
    cands = {float(r0)}
    lo = hi = r0
    for _ in range(2):
        lo = np.nextafter(lo, np.float32(-np.inf), dtype=np.float32)
        hi = np.nextafter(hi, np.float32(np.inf), dtype=np.float32)
        cands.update((float(lo), float(hi)))
    cands = sorted(cands)

    def gt(r):  # r > 1/sqrt(y)  <=>  r^2 * y > 1   (r > 0)
        return (Fraction(r) ** 2 * fy) > 1

    a = b = None
    for i in range(len(cands) - 1):
        if (not gt(cands[i])) and gt(cands[i + 1]):
            a, b = cands[i], cands[i + 1]
            break
    assert a is not None, "rsqrt bracket failure"
    m2 = Fraction(a + b) ** 2 * fy  # compare midpoint vs 1/sqrt(y)
    if m2 > 4:
        return np.float32(a)
    if m2 < 4:
        return np.float32(b)
    return np.float32(a) if (np.float32(a).view(np.int32) % 2 == 0) else np.float32(b)


def _thresholds(gamma, beta, running_mean, running_var) -> np.ndarray:
    """Per-channel U so that (x > U[c]) reproduces the reference's
    binarization decision bit-exactly.

    The reference (XLA, fma-contracted) binarizes +1 iff
        fl32(fma(fl32(x - mean), s, beta)) > 0.5,   s = fl32(gamma * rsqrt(var+eps))
    which, 0.5 being representable and ties rounding to even (0.5's mantissa),
    is exactly:  t1*s + beta > 1/2 + 2^-25 in exact arithmetic, t1 = fl32(x-mean).
    """
    yv = (running_var + BN_EPS).astype(np.float32)
    inv = np.array([_cr_rsqrt_f32(v) for v in yv], dtype=np.float32)
    s = (gamma * inv).astype(np.float32)
    M = Fraction(1, 2) + Fraction(1, 2 ** 25)

    U = np.zeros(C, dtype=np.float32)
    for c in range(C):
        sc, bc, mc = s[c], beta[c], running_mean[c]
        assert sc > 0, "threshold fold assumes positive BN scale"
        fs, fb = Fraction(float(sc)), Fraction(float(bc))

        def dec(xv):
            t1 = np.float32(xv) - mc
            return Fraction(float(t1)) * fs + fb > M

        xv = np.float32(np.float64(mc) + (0.5 - np.float64(bc)) / np.float64(sc))
        guard = 0
        while dec(xv):
            xv = np.nextafter(xv, np.float32(-np.inf), dtype=np.float32)
            guard += 1
            assert guard < 10000, "threshold search diverged"
        nxt = np.nextafter(xv, np.float32(np.inf), dtype=np.float32)
        while not dec(nxt):
            xv = nxt
            nxt = np.nextafter(xv, np.float32(np.inf), dtype=np.float32)
            guard += 1
            assert guard < 10000, "threshold search diverged"
        U[c] = xv  # largest fp32 x that binarizes to -1:  device does x > U
    return U


def _prep_inputs(x, gamma, beta, running_mean, running_var, W):
    U = _thresholds(
        np.asarray(gamma, dtype=np.float32),
        np.asarray(beta, dtype=np.float32),
        np.asarray(running_mean, dtype=np.float32),
        np.asarray(running_var, dtype=np.float32),
    )
    uv_dev = np.ascontiguousarray(U.reshape(2, 128).T)  # [p, cc]

    # wt[p, b, co] = fp16(2*W[co, cc*128+p, kh, kw]),  b = (cc*3 + kh)*3 + kw
    w2 = (np.asarray(W, dtype=np.float32) * np.float32(2.0)).astype(np.float16)
    wr = w2.reshape(C, 2, 128, KH, KW)
    wt_dev = wr.transpose(2, 1, 3, 4, 0).reshape(128, NB * C)
    # one combined constant tensor: weights ++ bit-packed U thresholds
    cw_dev = np.ascontiguousarray(
        np.concatenate([wt_dev, uv_dev.view(np.float16)], axis=1))

    x = np.ascontiguousarray(np.asarray(x, dtype=np.float32))
    in_maps = [
        {"xs": x[i * IMGS:(i + 1) * IMGS], "cw": cw_dev}
        for i in range(NCORES)
    ]
    return in_maps


def _run(in_maps, trace=False, **kwargs):
    return run_bass_kernel_spmd(
        _get_nc(), in_maps, list(range(NCORES)), trace=trace, **kwargs)


def kernel(x, gamma, beta, running_mean, running_var, W):
    in_maps = _prep_inputs(x, gamma, beta, running_mean, running_var, W)
    res = _run(in_maps)
    return np.concatenate([res.results[i]["y"] for i in range(NCORES)], axis=0)


# revision 22
# speedup vs baseline: 1.1223x; 1.0016x over previous
"""BinConv2d (BatchNorm -> BinActive -> pad(-1) -> 3x3 conv) on 8 TRN2 NeuronCores.

Strategy
--------
Data-parallel over the batch dim: 32 images -> 4 per core; BN params and conv
weights replicated.

The whole BN+binactive chain collapses into a per-channel fp32 threshold U[c]
computed on the host with exact rational arithmetic so that
    x > U[c]  <=>  round(clip((x-mean)*gamma*rsqrt(var+eps)+beta, 0, 1)) == 1
bit-for-bit as XLA (cpu and neuron backends agree bitwise: the mul+add is
FMA-contracted, rsqrt is the correctly-rounded 1/sqrt).

On device each input tile needs ONE vector op:
    xb = (x > U[c]) - 0.5  in {-0.5, +0.5}, cast to fp16
and the conv weights are host-doubled (2W in fp16), so (+-0.5)*(2w) = +-w
exactly (powers of two).  The conv itself is an implicit GEMM: for each
(image, co-chunk, 8-row tile) a PSUM tile [128 co, 448] accumulates 18
matmuls (2 ci-chunks x 9 taps) with fp16 operands at full PE rate.
"""

from fractions import Fraction

import numpy as np

import concourse.bass as bass
import concourse.mybir as mybir
from concourse.bass_utils import run_bass_kernel_spmd
from concourse.tile import TileContext

N, C, H, W_ = 32, 256, 56, 56
NCORES = 8
IMGS = N // NCORES          # 4 images per core
KH = KW = 3
ROWS = 8                    # output rows per matmul tile
NRT = H // ROWS             # 7 row tiles
NB = 2 * KH * KW            # 18 accumulation steps (ci-chunk x tap)
FREE = ROWS * W_            # 448 (<=512 fp32 PSUM bank)
BN_EPS = np.float32(1e-4)

_NC = None


def _legalize_waits(nc):
    """The TRN2 ISA takes ONE sync-wait per instruction, but Tile's wait
    assignment can attach several (walrus rejects with 'Too many sync wait
    commands').  Split the extras into preceding same-engine NoOps, each
    carrying a single wait — semantically identical (engine streams are
    in-order)."""
    k = 0
    for fn in nc.m.functions:
        for blk in fn.blocks:
            new_insts = []
            for inst in blk.instructions:
                si = inst.sync_info
                waits = list(si.on_wait) if si and si.on_wait else []
                if len(waits) > 1:
                    for w in waits[:-1]:
                        nop = mybir.InstNoOp(name=f"waitsplit-{k}")
                        k += 1
                        nop.engine = inst.engine
                        nop.bass_nofuse = True
                        nop.sync_info = mybir.SyncInfo(on_wait=[w], on_update=[])
                        new_insts.append(nop)
                    inst.sync_info = mybir.SyncInfo(
                        on_wait=[waits[-1]],
                        on_update=list(si.on_update) if si.on_update else [])
                new_insts.append(inst)
            blk.instructions = new_insts


def _build_nc():
    nc = bass.Bass("TRN2")
    xs = nc.dram_tensor("xs", [IMGS, C, H, W_], mybir.dt.float32, kind="ExternalInput")
    # weights [p, b*co] with the U thresholds bit-packed at the tail ->
    # ONE 9224B descriptor per partition for all constants
    cw = nc.dram_tensor("cw", [128, NB * C + 4], mybir.dt.float16,
                        kind="ExternalInput")
    y = nc.dram_tensor("y", [IMGS, C, H, W_], mybir.dt.float32, kind="ExternalOutput")

    with TileContext(nc) as tc:
        with (
            tc.tile_pool(name="const", bufs=1) as constp,
            tc.tile_pool(name="xpb", bufs=IMGS * 2) as xpbp,
            tc.tile_pool(name="xin", bufs=4) as xinp,
            tc.tile_pool(name="out", bufs=6) as outp,
            tc.tile_pool(name="ps", bufs=6, space="PSUM") as psp,
            tc.tile_pool(name="warm", bufs=1, space="PSUM") as warmp,
        ):
            # warm the PE clock (HAM) on a dependency-free junk tile so the
            # ramp happens during the framework preamble, not on the
            # critical path behind the weight DMA
            junk = constp.tile([128, 448], mybir.dt.float16, tag="junk")
            nc.gpsimd.memset(junk[:], 0.25)
            wps = warmp.tile([128, 448], mybir.dt.float32, tag="warm")
            for i in range(16):
                nc.tensor.matmul(wps[:], lhsT=junk[:, 0:128], rhs=junk[:],
                                 start=True, stop=True)

            cw_sb = constp.tile([128, NB * C + 4], mybir.dt.float16, tag="cw")
            nc.sync.dma_start(out=cw_sb[:], in_=cw[:])
            w_sb = cw_sb[:, :NB * C].rearrange("p (b c) -> p b c", c=C)
            uv_sb = cw_sb[:, NB * C:NB * C + 4].bitcast(mybir.dt.float32)

            def binarize(pb, xt_sl, cc, r0, r1):
                nc.vector.tensor_scalar(
                    out=pb[:, 1 + r0:1 + r1, 1:W_ + 1],
                    in0=xt_sl,
                    scalar1=uv_sb[:, cc:cc + 1],
                    scalar2=0.5,
                    op0=mybir.AluOpType.is_gt,
                    op1=mybir.AluOpType.subtract,
                )

            def borders(pb):
                # border pad = -0.5; on the DVE so ordering with the
                # interior write is program-order (no semaphores).
                # two strided memsets: rows {0,57}, then cols {0,57}
                nc.vector.memset(pb[:, 0:H + 2:H + 1, :], -0.5)
                nc.vector.memset(pb[:, 1:H + 1, 0:W_ + 2:W_ + 1], -0.5)

            # binarize each (image, ci-chunk) into a -0.5-padded fp16 buffer.
            # img0 streams in 8-row chunks (alternating ci-chunks) so the
            # first matmul group can start long before the full image lands.
            xpb = [[None] * 2 for _ in range(IMGS)]
            img0 = []
            for cc in range(2):
                xt = xinp.tile([128, H, W_], mybir.dt.float32, tag="xin")
                pb = xpbp.tile([128, H + 2, W_ + 2], mybir.dt.float16, tag="xpb")
                borders(pb)
                img0.append((xt, pb))
                xpb[0][cc] = pb
            # chunk 0 is 10 rows: exactly what the first matmul group's taps
            # touch (output rows 0-7 read padded rows 0-9), so the first
            # group fires one chunk earlier
            bounds = [0, ROWS + 2] + [k * ROWS + 2 for k in range(2, NRT)] + [H]
            assert bounds[-1] == H and all(
                b - a <= ROWS + 2 for a, b in zip(bounds, bounds[1:]))
            for r0, r1 in zip(bounds, bounds[1:]):
                for cc, (xt, pb) in enumerate(img0):
                    nc.sync.dma_start(
                        out=xt[:, r0:r1, :],
                        in_=xs[0, cc * 128:(cc + 1) * 128, r0:r1, :])
                    binarize(pb, xt[:, r0:r1, :], cc, r0, r1)

            def load_img(img):
                for cc in range(2):
                    xt = xinp.tile([128, H, W_], mybir.dt.float32, tag="xin")
                    nc.sync.dma_start(out=xt[:], in_=xs[img, cc * 128:(cc + 1) * 128])
                    pb = xpbp.tile([128, H + 2, W_ + 2], mybir.dt.float16, tag="xpb")
                    binarize(pb, xt[:], cc, 0, H)
                    borders(pb)
                    xpb[img][cc] = pb

            for img in range(IMGS):
                # emit the next image's load+binarize first so its DMAs are
                # enqueued ahead of this image's output stores
                if img + 1 < IMGS:
                    load_img(img + 1)
                for coj in range(2):
                    for rt in range(NRT):
                        ps = psp.tile([128, FREE], mybir.dt.float32, tag="ps")
                        for b in range(NB):
                            cc, t = divmod(b, KH * KW)
                            kh, kw = divmod(t, KW)
                            r = rt * ROWS + kh
                            nc.tensor.matmul(
                                ps[:],
                                lhsT=w_sb[:, b, coj * 128:(coj + 1) * 128],
                                rhs=xpb[img][cc][:, r:r + ROWS, kw:kw + W_],
                                start=(b == 0),
                                stop=(b == NB - 1),
                            )
                        ot = outp.tile([128, FREE], mybir.dt.float32, tag="ot")
                        nc.scalar.copy(out=ot[:], in_=ps[:])
                        nc.sync.dma_start(
                            out=y[img, coj * 128:(coj + 1) * 128,
                                  rt * ROWS:(rt + 1) * ROWS, :],
                            in_=ot[:],
                        )
    return nc


def _get_nc():
    global _NC
    if _NC is None:
        _NC = _build_nc()
        _legalize_waits(_NC)
    return _NC


def _cr_rsqrt_f32(yv: np.float32) -> np.float32:
    """Correctly-rounded fp32 1/sqrt(y) (round-to-nearest-even) — bitwise
    identical to XLA's rsqrt on both the cpu and neuron backends."""
    fy = Fraction(float(yv))
    r0 = np.float32(1.0 / np.sqrt(float(yv)))
    cands = {float(r0)}
    lo = hi = r0
    for _ in range(2):
        lo = np.nextafter(lo, np.float32(-np.inf), dtype=np.float32)
        hi = np.nextafter(hi, np.float32(np.inf), dtype=np.float32)
        cands.update((float(lo), float(hi)))
    cands = sorted(cands)

    def gt(r):  # r > 1/sqrt(y)  <=>  r^2 * y > 1   (r > 0)
        return (Fraction(r) ** 2 * fy) > 1

    a = b = None
    for i in range(len(cands) - 1):
        if (not gt(cands[i])) and gt(cands[i + 1]):
            a, b = cands[i], cands[i + 1]
            break
    assert a is not None, "rsqrt bracket failure"
    m2 = Fraction(a + b) ** 2 * fy  # compare midpoint vs 1/sqrt(y)
    if m2 > 4:
        return np.float32(a)
    if m2 < 4:
        return np.float32(b)
    return np.float32(a) if (np.float32(a).view(np.int32) % 2 == 0) else np.float32(b)


def _thresholds(gamma, beta, running_mean, running_var) -> np.ndarray:
    """Per-channel U so that (x > U[c]) reproduces the reference's
    binarization decision bit-exactly.

    The reference (XLA, fma-contracted) binarizes +1 iff
        fl32(fma(fl32(x - mean), s, beta)) > 0.5,   s = fl32(gamma * rsqrt(var+eps))
    which, 0.5 being representable and ties rounding to even (0.5's mantissa),
    is exactly:  t1*s + beta > 1/2 + 2^-25 in exact arithmetic, t1 = fl32(x-mean).
    """
    yv = (running_var + BN_EPS).astype(np.float32)
    inv = np.array([_cr_rsqrt_f32(v) for v in yv], dtype=np.float32)
    s = (gamma * inv).astype(np.float32)
    M = Fraction(1, 2) + Fraction(1, 2 ** 25)

    U = np.zeros(C, dtype=np.float32)
    for c in range(C):
        sc, bc, mc = s[c], beta[c], running_mean[c]
        assert sc > 0, "threshold fold assumes positive BN scale"
        fs, fb = Fraction(float(sc)), Fraction(float(bc))

        def dec(xv):
            t1 = np.float32(xv) - mc
            return Fraction(float(t1)) * fs + fb > M

        xv = np.float32(np.float64(mc) + (0.5 - np.float64(bc)) / np.float64(sc))
        guard = 0
        while dec(xv):
            xv = np.nextafter(xv, np.float32(-np.inf), dtype=np.float32)
            guard += 1
            assert guard < 10000, "threshold search diverged"
        nxt = np.nextafter(xv, np.float32(np.inf), dtype=np.float32)
        while not dec(nxt):
            xv = nxt
            nxt = np.nextafter(xv, np.float32(np.inf), dtype=np.float32)
            guard += 1
            assert guard < 10000, "threshold search diverged"
        U[c] = xv  # largest fp32 x that binarizes to -1:  device does x > U
    return U


def _prep_inputs(x, gamma, beta, running_mean, running_var, W):
    U = _thresholds(
        np.asarray(gamma, dtype=np.float32),
        np.asarray(beta, dtype=np.float32),
        np.asarray(running_mean, dtype=np.float32),
        np.asarray(running_var, dtype=np.float32),
    )
    uv_dev = np.ascontiguousarray(U.reshape(2, 128).T)  # [p, cc]

    # wt[p, b, co] = fp16(2*W[co, cc*128+p, kh, kw]),  b = (cc*3 + kh)*3 + kw
    w2 = (np.asarray(W, dtype=np.float32) * np.float32(2.0)).astype(np.float16)
    wr = w2.reshape(C, 2, 128, KH, KW)
    wt_dev = wr.transpose(2, 1, 3, 4, 0).reshape(128, NB * C)
    # one combined constant tensor: weights ++ bit-packed U thresholds
    cw_dev = np.ascontiguousarray(
        np.concatenate([wt_dev, uv_dev.view(np.float16)], axis=1))

    x = np.ascontiguousarray(np.asarray(x, dtype=np.float32))
    in_maps = [
        {"xs": x[i * IMGS:(i + 1) * IMGS], "cw": cw_dev}
        for i in range(NCORES)
    ]
    return in_maps


def _run(in_maps, trace=False, **kwargs):
    return run_bass_kernel_spmd(
        _get_nc(), in_maps, list(range(NCORES)), trace=trace, **kwargs)


def kernel(x, gamma, beta, running_mean, running_var, W):
    in_maps = _prep_inputs(x, gamma, beta, running_mean, running_var, W)
    res = _run(in_maps)
    return np.concatenate([res.results[i]["y"] for i in range(NCORES)], axis=0)
